# revision 9
# baseline (speedup 1.0000x reference)
"""Binary conv forward kernel for Trainium2 (8 NeuronCores, data-parallel over batch).

Computes y = conv2d(sign(x), scale[o] * sign(w)), stride 1, pad 1, NCHW/OIHW,
x [16, 64, 224, 224] f32, w [64*64*3*3, 1] f32 -> y [16, 64, 224, 224] f32.

Sharding: batch 16 -> 2 images per core, weights replicated (tiny).

HBM traffic is the roofline, so both directions are compressed:
  - Input: the host ships only the HIGH BYTE of each f32 (pure strided
    relayout, no arithmetic), packed [128, pair, batch, w].  Reinterpreted as
    fp8e4, that byte has the same sign as x (sign bit + top-7 exponent bits),
    so ScalarE's sign() recovers sign(x) exactly -- randn never produces
    |x| < 2^-126 (byte 0x00/0x80) or |x| >= 2^127 (NaN bytes).  4x fewer
    input bytes.
  - Output: written as fp16.  y = scale[o] * S with S an integer in
    [-576, 576]; fp16 rounding error is <= 2^-11 relative, far inside the
    2e-2 gate.  2x fewer output bytes.  Host upcasts to f32 on unpack.

Device algorithm (per core, n_batch=2 images):
  - A resident fp8 sign plane [128, NV+1, 464]: slot j = image rows (2j, 2j+1)
    (even row on partitions 0..63, odd on 64..127), both batch images in the
    free dim with one shared zero column between and zero pads at the edges so
    the kw shifts are exact.  Slot NV = copy of slot 0 (for the row-0/223
    boundary unit).  ScalarE signs 8 rows per ACTIVATE to amortize overhead.
  - Interior output pair (2m+1, 2m+2) accumulates in one PSUM bank via 3
    DoubleRow matmuls (virtual K=256 over slots m, m+1; M=128; N=450), one per
    kw shift.  Stationary blocks [[W0,0],[W1,W0]] / [[W2,W1],[0,W2]], where
    Wk = sign(w)[:,:,kh=k,kw]^T.  Boundary rows 0 and 223 use slots NV-1 and
    NV with blocks [[0,W0],[0,W1]] / [[W1,0],[W2,0]].
  - PSUM tiles span 4 banks = 4 units; eviction multiplies by the
    per-partition scale[o] (computed on device from raw weights) and writes
    fp16: one VectorE op per 4 units (last 3 groups ride ScalarE to balance
    engine load).
  - Input DMAs ride the HWDGE ring (nc.sync); weight + output DMAs ride SWDGE
    (nc.gpsimd) so loads and stores overlap on HBM.
"""

import numpy as np
import ml_dtypes

import concourse.bacc as bacc
import concourse.mybir as mybir
import concourse.tile as tile

F32 = mybir.dt.float32
F16 = mybir.dt.float16
FP8 = mybir.dt.float8e4

N_CORES = 8
FULL_BATCH = 16
C = 64  # in channels == out channels
H = 224
W = 224
KH = KW = 3
NV = H // 2  # row-pair slots / output units per image pair
# Sign-plane slot layout (fp8, per partition): [pad, b0 w=224, sep, b1 w=224,
# pad, pad] = 452 cols used, padded to 464 (multiple of 16 for DoubleRow AP
# steps).
SW = 464   # slot stride
SN = 450   # matmul N (448 real output columns + 2 junk)

GU = 4           # units per PSUM tile (4 banks)
OG = 8           # units per output chunk
# PSUM groups evicted on ScalarE to offload VectorE (even groups only; the
# eviction is emitted one group late so ScalarE reaches it with the matmuls
# already done -- no head-of-line stall of the sign stream).
ACT_EVICT_GROUPS = (18, 20, 22, 24)
# DoubleRowSwInterleave: host pre-interleaves the stationary pairs so
# LDWEIGHTS reads contiguously (raw DoubleRow pays +72% on the weight load).
SWI = True


def build_nc(n_batch=2):
    """Build the single-core Bass module (same NEFF runs on all 8 cores)."""
    nc = bacc.Bacc("TRN2", target_bir_lowering=False, debug=False)

    w = W
    xb = nc.dram_tensor("xb", [128, NV, n_batch, w], FP8, kind="ExternalInput")
    wraw = nc.dram_tensor("wraw", [C * C * KH * KW, 1], F32, kind="ExternalInput")
    # wblk: host-arranged raw f32 weights in the 6-tile DoubleRow block layout
    # [128, (3 interior + 3 boundary) tiles, 2, 128] with zeros in the zero
    # blocks (pure replication/padding; sign runs on device, sign(0) = 0).
    wblk = nc.dram_tensor("wblk", [128, 6 * 2 * 128], F32, kind="ExternalInput")
    yb = nc.dram_tensor("yb", [128, NV, n_batch, w], F16, kind="ExternalOutput")

    xr = xb.ap().rearrange("p j b w -> p j (b w)")   # [128, NV, 448]
    yr = yb.ap().rearrange("p j b w -> p j (b w)")

    # Input chunk schedule: small leading chunks so signing (and the first
    # matmuls) start as early as possible, then full chunks.
    sizes = [4, 8] + [16] * 6 + [4]
    assert sum(sizes) == NV
    starts = list(np.cumsum([0] + sizes[:-1]))

    # PSUM groups: 27 x 4 interior units, then [108, 109, 110, boundary].
    n_groups = 28

    with tile.TileContext(nc) as tc:
        with (
            tc.tile_pool(name="wpool", bufs=1) as wpool,
            tc.tile_pool(name="icpool", bufs=3) as icpool,
            tc.tile_pool(name="pspool", bufs=2, space="PSUM") as pspool,
            tc.tile_pool(name="ocpool", bufs=2) as ocpool,
        ):
            # ---- weight prep (one-time, tiny) ----
            # SWDGE (gpsimd) DMAs start executing ~6us before the HWDGE ring
            # warms up, so the weight block (which gates the first matmul) and
            # the input chunks ride gpsimd; the tiny scale input and all
            # output stores ride the sync ring.
            wblkf = wpool.tile([128, 6, 2, 128], F32)
            nc.gpsimd.dma_start(
                wblkf[:], wblk.ap().rearrange("p (t i m) -> p t i m", t=6, i=2)
            )
            sblk = wpool.tile([128, 6, 2, 128], FP8)
            nc.scalar.sign(sblk[:], wblkf[:])
            wdr = [sblk[:, kw, :, :] for kw in range(3)]
            wb = [sblk[:, 3 + kw, :, :] for kw in range(3)]

            # scale[o] = mean(|w[o]|), O on partitions, duplicated on both
            # partition halves for the [128]-row eviction.
            w2 = wpool.tile([128, 576], F32)
            wr = wraw.ap().rearrange("(o f) one -> o (f one)", o=C)
            nc.sync.dma_start(w2[0:64], wr)
            nc.sync.dma_start(w2[64:128], wr)
            absw = wpool.tile([128, 576], F32)
            sc_sum = wpool.tile([128, 1], F32)
            sc128 = wpool.tile([128, 1], F32)

            # Resident sign plane; slot NV = V_0 copy.  Zero the pad columns
            # once (plane slots are written exactly once).
            assert n_batch == 2
            plane = wpool.tile([128, NV + 1, SW], FP8)
            nc.vector.memset(plane[:, :, 0:1], 0.0)
            nc.vector.memset(plane[:, :, 225:226], 0.0)
            nc.vector.memset(plane[:, :, 450:452], 0.0)

            def rhs(j, kw):
                return plane[:, j : j + 2, kw : kw + SN]

            def emit_sign(ic, c0, r0, k):
                """Sign rows r0..r0+k (absolute) from chunk starting at c0."""
                nc.scalar.sign(
                    plane[:, r0 : r0 + k, 1:451].rearrange(
                        "p j (b w) -> p j b w", w=225
                    )[:, :, :, 0:w],
                    ic[:, r0 - c0 : r0 - c0 + k, :].rearrange(
                        "p j (b w) -> p j b w", b=n_batch
                    ),
                )

            def evict(engine, ps, oc, oslot):
                out_ap = oc[:, oslot : oslot + GU, :].rearrange(
                    "p u (b w) -> p u b w", b=n_batch
                )
                in_ap = ps[:, 0:GU, 0:450].rearrange(
                    "p u (b w) -> p u b w", w=225
                )[:, :, :, 0:w]
                if engine == "act":
                    nc.scalar.mul(out_ap, in_ap, sc128[:])
                else:
                    nc.vector.tensor_scalar_mul(out_ap, in_ap, sc128[:])

            # ---- main pipeline ----
            next_chunk = 0   # next input chunk to DMA
            rows_avail = 0   # rows resident in SBUF (DMA emitted)
            rows_signed = 0  # rows signed into the plane
            ic = None
            ic_c0 = 0

            def ensure_signed(upto):
                """Emit chunk DMAs + sign ops until rows [0, upto) signed."""
                nonlocal next_chunk, rows_avail, rows_signed, ic, ic_c0
                while rows_signed < upto:
                    if rows_signed == rows_avail:
                        gc = sizes[next_chunk]
                        c0 = starts[next_chunk]
                        ic = icpool.tile([128, 16, n_batch * w], FP8, tag="ic")
                        nc.gpsimd.dma_start(ic[:, 0:gc, :], xr[:, c0 : c0 + gc, :])
                        ic_c0 = c0
                        rows_avail += gc
                        next_chunk += 1
                    k = min(16, rows_avail - rows_signed)
                    emit_sign(ic, ic_c0, rows_signed, k)
                    rows_signed += k
                    if rows_signed - k == 0:
                        # V_0 copy for the boundary unit + scale prep, right
                        # after the first sign lands.
                        nc.vector.tensor_copy(
                            out=plane[:, NV, 0:452], in_=plane[:, 0, 0:452]
                        )
                        nc.scalar.activation(
                            out=absw[:], in_=w2[:],
                            func=mybir.ActivationFunctionType.Abs,
                            accum_out=sc_sum[:],
                        )
                        nc.scalar.mul(sc128[:], sc_sum[:], 1.0 / 576.0)

            pm = (
                mybir.MatmulPerfMode.DoubleRowSwInterleave
                if SWI
                else mybir.MatmulPerfMode.DoubleRow
            )
            oc = None
            pending_act = None  # (ps, oc, oslot) evicted on ScalarE next group
            for g in range(n_groups):
                m0 = GU * g
                boundary = g == n_groups - 1
                nu = GU - 1 if boundary else GU  # interior units in group
                ensure_signed(min(m0 + nu + 1, NV))

                # Flush the previous group's deferred ScalarE eviction now:
                # its matmuls depended on an older sign batch, so ScalarE
                # reaches it with the PSUM data already in place (no
                # head-of-line stall of the sign stream), and it still
                # precedes the g+1 matmuls that reuse the PSUM buffer.
                if pending_act is not None:
                    evict("act", *pending_act)
                    pending_act = None

                if m0 % OG == 0:
                    oc = ocpool.tile([128, OG, n_batch * w], F16, tag="oc")

                ps = pspool.tile([128, GU, 512], F32, tag="ps")
                for u in range(nu):
                    m = m0 + u
                    for kw in range(3):
                        nc.tensor.matmul(
                            ps[:, u, 0:SN], wdr[kw][:], rhs(m, kw),
                            start=(kw == 0), stop=(kw == 2),
                            perf_mode=pm,
                        )
                if boundary:
                    for kw in range(3):
                        nc.tensor.matmul(
                            ps[:, GU - 1, 0:SN], wb[kw][:], rhs(NV - 1, kw),
                            start=(kw == 0), stop=(kw == 2),
                            perf_mode=pm,
                        )

                if g in ACT_EVICT_GROUPS:
                    pending_act = (ps, oc, m0 % OG)
                else:
                    evict("dve", ps, oc, m0 % OG)

                if m0 % OG == GU or boundary:
                    base = (m0 // OG) * OG
                    nc.gpsimd.dma_start(
                        yr[:, base : base + OG, :], oc[:, 0:OG, :]
                    )

    nc.compile()
    return nc


_NC_CACHE = {}


def _get_nc(key=(2,)):
    if key not in _NC_CACHE:
        _NC_CACHE[key] = build_nc(*key)
    return _NC_CACHE[key]


def _make_wblk(weights):
    """Arrange raw f32 weights into the 6-tile DoubleRow block layout
    [128, 6, 2, 128] (pure replication/zero-padding; sign runs on device)."""
    wt = weights.reshape(C, C, KH, KW).transpose(1, 2, 3, 0)  # [i, kh, kw, o]

    def T(kh, kw):
        return wt[:, kh, kw, :]  # W_{kh,kw}^T as [i, o]

    blk = np.zeros((128, 6, 2, 128), np.float32)
    for kw in range(KW):
        # interior tiles: i=0 -> [[W0, 0], [W1, W0]], i=1 -> [[W2, W1], [0, W2]]
        blk[0:64, kw, 0, 0:64] = T(0, kw)
        blk[64:128, kw, 0, 0:64] = T(1, kw)
        blk[64:128, kw, 0, 64:128] = T(0, kw)
        blk[0:64, kw, 1, 0:64] = T(2, kw)
        blk[0:64, kw, 1, 64:128] = T(1, kw)
        blk[64:128, kw, 1, 64:128] = T(2, kw)
        # boundary tiles: i=0 -> [[0, W0], [0, W1]], i=1 -> [[W1, 0], [W2, 0]]
        blk[0:64, 3 + kw, 0, 64:128] = T(0, kw)
        blk[64:128, 3 + kw, 0, 64:128] = T(1, kw)
        blk[0:64, 3 + kw, 1, 0:64] = T(1, kw)
        blk[64:128, 3 + kw, 1, 0:64] = T(2, kw)
    if SWI:
        # DoubleRowSwInterleave layout: per stationary tile, pairs (A, B)
        # interleaved per output column, columns reversed:
        # [A_127, B_127, A_126, B_126, ..., A_0, B_0].
        swi = np.zeros((128, 6, 256), np.float32)
        swi[:, :, 0::2] = blk[:, :, 0, ::-1]
        swi[:, :, 1::2] = blk[:, :, 1, ::-1]
        return swi.reshape(128, 6 * 2 * 128)
    return blk.reshape(128, 6 * 2 * 128)


def pack_x(x_shard):
    """f32 [nb, C, h, w] -> high-byte plane [128, NV, nb, w] (fp8e4 view);
    p = parity*64 + channel.  Pure strided relayout of the sign/exponent
    byte -- no arithmetic."""
    nb = x_shard.shape[0]
    hb = x_shard.view(np.uint8).reshape(nb, C, NV, 2, W, 4)[..., 3]
    packed = np.ascontiguousarray(hb.transpose(3, 1, 2, 0, 4)).reshape(
        128, NV, nb, W
    )
    return packed.view(ml_dtypes.float8_e4m3fn)


def unpack_y(ypk):
    """fp16 [128, NV, nb, w] -> f32 [nb, C, h, w] per the unit layout."""
    nb = ypk.shape[2]
    y = np.empty((nb, C, H, W), np.float32)
    # interior units m=0..NV-2 -> rows 2m+1 (p<64) and 2m+2 (p>=64)
    y[:, :, 1 : H - 1 : 2] = ypk[0:C, 0 : NV - 1].transpose(2, 0, 1, 3)
    y[:, :, 2 : H : 2] = ypk[C:128, 0 : NV - 1].transpose(2, 0, 1, 3)
    # boundary unit: p<64 -> row 0, p>=64 -> row H-1
    y[:, :, 0] = ypk[0:C, NV - 1].transpose(1, 0, 2)
    y[:, :, H - 1] = ypk[C:128, NV - 1].transpose(1, 0, 2)
    return y


def make_in_maps(x, weights):
    x = np.ascontiguousarray(np.asarray(x, dtype=np.float32))
    weights = np.asarray(weights, dtype=np.float32)
    wblk = _make_wblk(weights)
    nb = FULL_BATCH // N_CORES
    return [
        {
            "xb": pack_x(x[c * nb : (c + 1) * nb]),
            "wraw": weights,
            "wblk": wblk,
        }
        for c in range(N_CORES)
    ]


def gather_out(results):
    return np.concatenate([unpack_y(r["yb"]) for r in results], axis=0)


def kernel(x, weights):
    from concourse import bass_utils

    nc = _get_nc()
    in_maps = make_in_maps(x, weights)
    res = bass_utils.run_bass_kernel_spmd(nc, in_maps, core_ids=list(range(N_CORES)))
    return gather_out(res.results)


# revision 11
# speedup vs baseline: 1.1696x; 1.1696x over previous
"""Binary conv forward kernel for Trainium2 (8 NeuronCores, data-parallel over batch).

Computes y = conv2d(sign(x), scale[o] * sign(w)), stride 1, pad 1, NCHW/OIHW,
x [16, 64, 224, 224] f32, w [64*64*3*3, 1] f32 -> y [16, 64, 224, 224] f32.

Sharding: batch 16 -> 2 images per core, weights replicated (tiny).

HBM traffic is the roofline, so both directions are compressed:
  - Input: the host ships only the HIGH BYTE of each f32 (pure strided
    relayout, no arithmetic), packed [128, pair, batch, w].  Reinterpreted as
    fp8e4, that byte has the same sign as x (sign bit + top-7 exponent bits),
    so ScalarE's sign() recovers sign(x) exactly -- randn never produces
    |x| < 2^-126 (byte 0x00/0x80) or |x| >= 2^127 (NaN bytes).  4x fewer
    input bytes.
  - Output: written as fp16.  y = scale[o] * S with S an integer in
    [-576, 576]; fp16 rounding error is <= 2^-11 relative, far inside the
    2e-2 gate.  2x fewer output bytes.  Host upcasts to f32 on unpack.

Device algorithm (per core, n_batch=2 images):
  - A resident fp8 sign plane [128, NV+1, 464]: slot j = image rows (2j, 2j+1)
    (even row on partitions 0..63, odd on 64..127), both batch images in the
    free dim with one shared zero column between and zero pads at the edges so
    the kw shifts are exact.  Slot NV = copy of slot 0 (for the row-0/223
    boundary unit).  ScalarE signs 8 rows per ACTIVATE to amortize overhead.
  - Interior output pair (2m+1, 2m+2) accumulates in one PSUM bank via 3
    DoubleRow matmuls (virtual K=256 over slots m, m+1; M=128; N=450), one per
    kw shift.  Stationary blocks [[W0,0],[W1,W0]] / [[W2,W1],[0,W2]], where
    Wk = sign(w)[:,:,kh=k,kw]^T.  Boundary rows 0 and 223 use slots NV-1 and
    NV with blocks [[0,W0],[0,W1]] / [[W1,0],[W2,0]].
  - PSUM tiles span 4 banks = 4 units; eviction multiplies by the
    per-partition scale[o] (computed on device from raw weights) and writes
    fp16: one VectorE op per 4 units (last 3 groups ride ScalarE to balance
    engine load).
  - Input DMAs ride the HWDGE ring (nc.sync); weight + output DMAs ride SWDGE
    (nc.gpsimd) so loads and stores overlap on HBM.
"""

import numpy as np
import ml_dtypes

import concourse.bacc as bacc
import concourse.mybir as mybir
import concourse.tile as tile

F32 = mybir.dt.float32
F16 = mybir.dt.float16
FP8 = mybir.dt.float8e4

N_CORES = 8
FULL_BATCH = 16
C = 64  # in channels == out channels
H = 224
W = 224
KH = KW = 3
NV = H // 2  # row-pair slots / output units per image pair
# Sign-plane slot layout (fp8, per partition): [pad, b0 w=224, sep, b1 w=224,
# pad, pad] = 452 cols used, padded to 464 (multiple of 16 for DoubleRow AP
# steps).
SW = 464   # slot stride
SN = 449   # matmul N (448 real output columns + 1 junk)

GU = 2           # units per PSUM tile (2 banks; bufs=4 -> PE runs 3 groups ahead)
OG = 8           # units per output chunk
ACT_EVICT_GROUPS = ()  # all evictions on VectorE (ScalarE is sign-bound)
# DoubleRowSwInterleave: host pre-interleaves the stationary pairs so
# LDWEIGHTS reads contiguously (raw DoubleRow pays +72% on the weight load).
SWI = False


def build_nc(n_batch=2):
    """Build the single-core Bass module (same NEFF runs on all 8 cores)."""
    nc = bacc.Bacc("TRN2", target_bir_lowering=False, debug=False)

    w = W
    xb = nc.dram_tensor("xb", [128, NV, n_batch, w], FP8, kind="ExternalInput")
    wraw = nc.dram_tensor("wraw", [C * C * KH * KW, 1], F32, kind="ExternalInput")
    # wblk: host-arranged raw f32 weights in the 6-tile DoubleRow block layout
    # [128, (3 interior + 3 boundary) tiles, 2, 128] with zeros in the zero
    # blocks (pure replication/padding; sign runs on device, sign(0) = 0).
    wblk = nc.dram_tensor("wblk", [128, 6 * 2 * 128], F32, kind="ExternalInput")
    yb = nc.dram_tensor("yb", [128, NV, n_batch, w], F16, kind="ExternalOutput")

    xr = xb.ap().rearrange("p j b w -> p j (b w)")   # [128, NV, 448]
    yr = yb.ap().rearrange("p j b w -> p j (b w)")

    # Input chunk schedule: small leading chunks so signing (and the first
    # matmuls) start as early as possible, then full chunks.
    sizes = [4, 4, 8] + [16] * 6
    assert sum(sizes) == NV
    starts = list(np.cumsum([0] + sizes[:-1]))

    # PSUM groups: 55 x 2 interior units, then [110, boundary].
    n_groups = 56

    with tile.TileContext(nc) as tc:
        with (
            tc.tile_pool(name="wpool", bufs=1) as wpool,
            tc.tile_pool(name="icpool", bufs=3) as icpool,
            tc.tile_pool(name="pspool", bufs=4, space="PSUM") as pspool,
            tc.tile_pool(name="ocpool", bufs=2) as ocpool,
        ):
            # ---- weight prep (one-time, tiny) ----
            # SWDGE (gpsimd) DMAs start executing ~6us before the HWDGE ring
            # warms up, so the weight block (which gates the first matmul) and
            # the input chunks ride gpsimd; the tiny scale input and all
            # output stores ride the sync ring.
            wblkf = wpool.tile([128, 6, 2, 128], F32)
            nc.gpsimd.dma_start(
                wblkf[:], wblk.ap().rearrange("p (t i m) -> p t i m", t=6, i=2)
            )
            sblk = wpool.tile([128, 6, 2, 128], FP8)
            nc.scalar.sign(sblk[:], wblkf[:])
            wdr = [sblk[:, kw, :, :] for kw in range(3)]
            wb = [sblk[:, 3 + kw, :, :] for kw in range(3)]

            # scale[o] = mean(|w[o]|), O on partitions, duplicated on both
            # partition halves for the [128]-row eviction.
            w2 = wpool.tile([128, 576], F32)
            wr = wraw.ap().rearrange("(o f) one -> o (f one)", o=C)
            nc.sync.dma_start(w2[0:64], wr)
            nc.sync.dma_start(w2[64:128], wr)
            absw = wpool.tile([128, 576], F32)
            sc_sum = wpool.tile([128, 1], F32)
            sc128 = wpool.tile([128, 1], F32)

            # Resident sign plane; slot NV = V_0 copy.  Zero the pad columns
            # once (plane slots are written exactly once).
            assert n_batch == 2
            plane = wpool.tile([128, NV + 1, SW], FP8)
            nc.vector.memset(plane[:, :, 0:1], 0.0)
            nc.vector.memset(plane[:, :, 225:226], 0.0)
            nc.vector.memset(plane[:, :, 450:452], 0.0)

            def rhs(j, kw):
                return plane[:, j : j + 2, kw : kw + SN]

            def emit_sign(ic, c0, r0, k):
                """Sign rows r0..r0+k (absolute) from chunk starting at c0."""
                nc.scalar.sign(
                    plane[:, r0 : r0 + k, 1:451].rearrange(
                        "p j (b w) -> p j b w", w=225
                    )[:, :, :, 0:w],
                    ic[:, r0 - c0 : r0 - c0 + k, :].rearrange(
                        "p j (b w) -> p j b w", b=n_batch
                    ),
                )

            def evict(engine, ps, oc, oslot):
                out_ap = oc[:, oslot : oslot + GU, :].rearrange(
                    "p u (b w) -> p u b w", b=n_batch
                )
                in_ap = ps[:, 0:GU, 0:450].rearrange(
                    "p u (b w) -> p u b w", w=225
                )[:, :, :, 0:w]
                if engine == "act":
                    nc.scalar.mul(out_ap, in_ap, sc128[:])
                else:
                    nc.vector.tensor_scalar_mul(out_ap, in_ap, sc128[:])

            # ---- main pipeline ----
            next_chunk = 0   # next input chunk to DMA
            rows_avail = 0   # rows resident in SBUF (DMA emitted)
            rows_signed = 0  # rows signed into the plane
            ic = None
            ic_c0 = 0

            def ensure_signed(upto):
                """Emit chunk DMAs + sign ops until rows [0, upto) signed."""
                nonlocal next_chunk, rows_avail, rows_signed, ic, ic_c0
                while rows_signed < upto:
                    if rows_signed == rows_avail:
                        gc = sizes[next_chunk]
                        c0 = starts[next_chunk]
                        ic = icpool.tile([128, 16, n_batch * w], FP8, tag="ic")
                        nc.sync.dma_start(ic[:, 0:gc, :], xr[:, c0 : c0 + gc, :])
                        ic_c0 = c0
                        rows_avail += gc
                        next_chunk += 1
                    k = min(8, rows_avail - rows_signed)
                    emit_sign(ic, ic_c0, rows_signed, k)
                    rows_signed += k
                    if rows_signed - k == 0:
                        # V_0 copy for the boundary unit + scale prep, right
                        # after the first sign lands.
                        nc.vector.tensor_copy(
                            out=plane[:, NV, 0:452], in_=plane[:, 0, 0:452]
                        )
                        nc.scalar.activation(
                            out=absw[:], in_=w2[:],
                            func=mybir.ActivationFunctionType.Abs,
                            accum_out=sc_sum[:],
                        )
                        nc.scalar.mul(sc128[:], sc_sum[:], 1.0 / 576.0)

            pm = (
                mybir.MatmulPerfMode.DoubleRowSwInterleave
                if SWI
                else mybir.MatmulPerfMode.DoubleRow
            )
            oc = None
            pending_act = None  # (ps, oc, oslot) evicted on ScalarE next group
            for g in range(n_groups):
                m0 = GU * g
                boundary = g == n_groups - 1
                nu = GU - 1 if boundary else GU  # interior units in group
                ensure_signed(min(m0 + nu + 1, NV))

                # Flush the previous group's deferred ScalarE eviction now:
                # its matmuls depended on an older sign batch, so ScalarE
                # reaches it with the PSUM data already in place (no
                # head-of-line stall of the sign stream), and it still
                # precedes the g+1 matmuls that reuse the PSUM buffer.
                if pending_act is not None:
                    evict("act", *pending_act)
                    pending_act = None

                if m0 % OG == 0:
                    oc = ocpool.tile([128, OG, n_batch * w], F16, tag="oc")

                ps = pspool.tile([128, GU, 512], F32, tag="ps")
                for u in range(nu):
                    m = m0 + u
                    for kw in range(3):
                        nc.tensor.matmul(
                            ps[:, u, 0:SN], wdr[kw][:], rhs(m, kw),
                            start=(kw == 0), stop=(kw == 2),
                            perf_mode=pm,
                        )
                if boundary:
                    for kw in range(3):
                        nc.tensor.matmul(
                            ps[:, GU - 1, 0:SN], wb[kw][:], rhs(NV - 1, kw),
                            start=(kw == 0), stop=(kw == 2),
                            perf_mode=pm,
                        )

                if g in ACT_EVICT_GROUPS:
                    pending_act = (ps, oc, m0 % OG)
                else:
                    evict("dve", ps, oc, m0 % OG)

                if m0 % OG == OG - GU or boundary:
                    base = (m0 // OG) * OG
                    nc.gpsimd.dma_start(
                        yr[:, base : base + OG, :], oc[:, 0:OG, :]
                    )

    nc.compile()
    return nc


_NC_CACHE = {}


def _get_nc(key=(2,)):
    if key not in _NC_CACHE:
        _NC_CACHE[key] = build_nc(*key)
    return _NC_CACHE[key]


def _make_wblk(weights):
    """Arrange raw f32 weights into the 6-tile DoubleRow block layout
    [128, 6, 2, 128] (pure replication/zero-padding; sign runs on device)."""
    wt = weights.reshape(C, C, KH, KW).transpose(1, 2, 3, 0)  # [i, kh, kw, o]

    def T(kh, kw):
        return wt[:, kh, kw, :]  # W_{kh,kw}^T as [i, o]

    blk = np.zeros((128, 6, 2, 128), np.float32)
    for kw in range(KW):
        # interior tiles: i=0 -> [[W0, 0], [W1, W0]], i=1 -> [[W2, W1], [0, W2]]
        blk[0:64, kw, 0, 0:64] = T(0, kw)
        blk[64:128, kw, 0, 0:64] = T(1, kw)
        blk[64:128, kw, 0, 64:128] = T(0, kw)
        blk[0:64, kw, 1, 0:64] = T(2, kw)
        blk[0:64, kw, 1, 64:128] = T(1, kw)
        blk[64:128, kw, 1, 64:128] = T(2, kw)
        # boundary tiles: i=0 -> [[0, W0], [0, W1]], i=1 -> [[W1, 0], [W2, 0]]
        blk[0:64, 3 + kw, 0, 64:128] = T(0, kw)
        blk[64:128, 3 + kw, 0, 64:128] = T(1, kw)
        blk[0:64, 3 + kw, 1, 0:64] = T(1, kw)
        blk[64:128, 3 + kw, 1, 0:64] = T(2, kw)
    if SWI:
        # DoubleRowSwInterleave layout: per stationary tile, pairs (A, B)
        # interleaved per output column, columns reversed:
        # [A_127, B_127, A_126, B_126, ..., A_0, B_0].
        swi = np.zeros((128, 6, 256), np.float32)
        swi[:, :, 0::2] = blk[:, :, 0, ::-1]
        swi[:, :, 1::2] = blk[:, :, 1, ::-1]
        return swi.reshape(128, 6 * 2 * 128)
    return blk.reshape(128, 6 * 2 * 128)


def pack_x(x_shard):
    """f32 [nb, C, h, w] -> high-byte plane [128, NV, nb, w] (fp8e4 view);
    p = parity*64 + channel.  Pure strided relayout of the sign/exponent
    byte -- no arithmetic."""
    nb = x_shard.shape[0]
    hb = x_shard.view(np.uint8).reshape(nb, C, NV, 2, W, 4)[..., 3]
    packed = np.ascontiguousarray(hb.transpose(3, 1, 2, 0, 4)).reshape(
        128, NV, nb, W
    )
    return packed.view(ml_dtypes.float8_e4m3fn)


def unpack_y(ypk):
    """fp16 [128, NV, nb, w] -> f32 [nb, C, h, w] per the unit layout."""
    nb = ypk.shape[2]
    y = np.empty((nb, C, H, W), np.float32)
    # interior units m=0..NV-2 -> rows 2m+1 (p<64) and 2m+2 (p>=64)
    y[:, :, 1 : H - 1 : 2] = ypk[0:C, 0 : NV - 1].transpose(2, 0, 1, 3)
    y[:, :, 2 : H : 2] = ypk[C:128, 0 : NV - 1].transpose(2, 0, 1, 3)
    # boundary unit: p<64 -> row 0, p>=64 -> row H-1
    y[:, :, 0] = ypk[0:C, NV - 1].transpose(1, 0, 2)
    y[:, :, H - 1] = ypk[C:128, NV - 1].transpose(1, 0, 2)
    return y


def make_in_maps(x, weights):
    x = np.ascontiguousarray(np.asarray(x, dtype=np.float32))
    weights = np.asarray(weights, dtype=np.float32)
    wblk = _make_wblk(weights)
    nb = FULL_BATCH // N_CORES
    return [
        {
            "xb": pack_x(x[c * nb : (c + 1) * nb]),
            "wraw": weights,
            "wblk": wblk,
        }
        for c in range(N_CORES)
    ]


def gather_out(results):
    return np.concatenate([unpack_y(r["yb"]) for r in results], axis=0)


def kernel(x, weights):
    from concourse import bass_utils

    nc = _get_nc()
    in_maps = make_in_maps(x, weights)
    res = bass_utils.run_bass_kernel_spmd(nc, in_maps, core_ids=list(range(N_CORES)))
    return gather_out(res.results)
